# revision 2
# baseline (speedup 1.0000x reference)
"""CCN promotion layer (gnn_message_passing) for 8x Trainium2 NeuronCores.

reference:
    new_parts = sort(neigh, axis=1)                      # [N, D]
    chi[n,c,a] = (new_parts[n,a] == neigh[n,c])          # [N, D, D]
    feat[n,c,s] = tensors[neigh[n,c], s]                 # [N, D, F]
    promotions[n,c,a,b,s] = chi[n,c,a]*chi[n,c,b]*feat[n,c,s]   # [N,D,D,D,F]

Because `neigh` rows are sorted (harness fill: sorted_randint), new_parts ==
neigh, so chi is the equality matrix of each row with itself: identity except
for duplicate-value runs.  Hence promotions is ~all zeros, with feat[n,c,:] at
the diagonal (c,c,c) slots, plus S^3 blocks for duplicate runs S (rare:
~0.6% of rows for D=16 draws out of N=20000).

Device strategy (memory-regime roofline = writing the ~983 MB output):
  - shard nodes across 8 cores (2500 nodes/core, 122.9 MB output/core)
  - per core, keep 3 persistent SBUF tiles [125 nodes, 16, 819] zeroed once;
    each loop iteration DMAs the 125x48 feature block in, copies it into the
    diagonal slots (stride 819 = 768+48+3), and DMAs the 125x12288 tile to
    its contiguous 6.14 MB slice of the output.  Only the diagonal slots are
    ever rewritten, so the zeros never need refreshing.
  - host fixes up the rare duplicate rows afterwards (a few KB of writes).
"""

import os
import sys

import numpy as np

N, D, F = 20000, 16, 3
NCORES = 8
NPC = N // NCORES                  # nodes per core: 2500
TILE_P = 125                       # nodes per SBUF tile (partition dim)
NTILES = NPC // TILE_P             # 20
ROW = D * D * D * F                # 12288 floats per node
CSTRIDE = D * D * F + D * F + F    # 819: flat stride between (c,c,c,0) slots
NBIGBUF = 3

_cache = {}


def _get_program():
    if "nc" in _cache:
        return _cache["nc"]
    if "/opt/trn_rl_repo" not in sys.path:
        sys.path.insert(0, "/opt/trn_rl_repo")
    from concourse import bacc, mybir, tile

    nc = bacc.Bacc("TRN2", target_bir_lowering=False, debug=False,
                   num_devices=NCORES)
    feat = nc.dram_tensor("feat", [NPC, D * F], mybir.dt.float32,
                          kind="ExternalInput").ap()
    out = nc.dram_tensor("out", [NPC, ROW], mybir.dt.float32,
                         kind="ExternalOutput").ap()

    with tile.TileContext(nc) as tc:
        with tc.tile_pool(name="bigpool", bufs=1) as bigpool, \
             tc.tile_pool(name="featpool", bufs=4) as featpool:
            bigs = []
            for i in range(NBIGBUF):
                big = bigpool.tile([TILE_P, D, CSTRIDE], mybir.dt.float32,
                                   name=f"big{i}", tag=f"big{i}")
                eng = nc.vector if i % 2 == 0 else nc.gpsimd
                eng.memset(big[:], 0.0)
                bigs.append(big)
            for t in range(NTILES):
                buf = bigs[t % NBIGBUF]
                fb = featpool.tile([TILE_P, D * F], mybir.dt.float32,
                                   name="fb", tag="fb")
                nc.sync.dma_start(
                    out=fb[:], in_=feat[t * TILE_P:(t + 1) * TILE_P, :])
                nc.vector.tensor_copy(
                    out=buf[:, :, 0:F],
                    in_=fb.rearrange("p (c s) -> p c s", c=D))
                flat = buf.rearrange("p c r -> p (c r)")[:, :ROW]
                nc.sync.dma_start(
                    out=out[t * TILE_P:(t + 1) * TILE_P, :], in_=flat)
    nc.compile()
    _cache["nc"] = nc
    return nc


def _host_reference(tensors, neigh):
    new_parts = np.sort(neigh, axis=1)
    chi = (new_parts[:, None, :] == neigh[:, :, None]).astype(np.float32)
    feat = tensors[neigh]
    promotions = (chi[:, :, :, None, None] * chi[:, :, None, :, None]
                  * feat[:, :, None, None, :]).astype(np.float32)
    return promotions, new_parts


def kernel(tensors, neigh):
    tensors = np.ascontiguousarray(np.asarray(tensors), dtype=np.float32)
    neigh_in = np.asarray(neigh)
    neigh_i = np.ascontiguousarray(neigh_in.astype(np.int64))

    sorted_ok = bool((neigh_i[:, 1:] >= neigh_i[:, :-1]).all())
    if (not sorted_ok or tensors.shape != (N, F)
            or neigh_i.shape != (N, D)):
        prom, parts = _host_reference(tensors, neigh_in)
        return prom, parts.astype(neigh_in.dtype)

    feat_flat = tensors[neigh_i].reshape(N, D * F)

    nc = _get_program()
    from concourse.bass_utils import run_bass_kernel_spmd

    in_maps = [{"feat": feat_flat[c * NPC:(c + 1) * NPC]}
               for c in range(NCORES)]
    trace = bool(os.environ.get("PROMO_TRACE"))
    res = run_bass_kernel_spmd(nc, in_maps, core_ids=list(range(NCORES)),
                               trace=trace)
    _cache["last_results"] = res

    promotions = np.empty((N, ROW), dtype=np.float32)
    for c in range(NCORES):
        promotions[c * NPC:(c + 1) * NPC] = res.results[c]["out"]
    promotions = promotions.reshape(N, D, D, D, F)

    # Duplicate-run fixup: for a run S of equal values in the sorted row,
    # promotions[n, c, a, b, :] = tensors[v] for all (c, a, b) in S^3.
    dup_rows = np.nonzero((neigh_i[:, 1:] == neigh_i[:, :-1]).any(axis=1))[0]
    for n in dup_rows:
        row = neigh_i[n]
        vals, starts, counts = np.unique(row, return_index=True,
                                         return_counts=True)
        for v, st, ct in zip(vals, starts, counts):
            if ct >= 2:
                S = np.arange(st, st + ct)
                promotions[n][np.ix_(S, S, S)] = tensors[v]

    new_parts = np.sort(neigh_in, axis=1)
    return promotions, new_parts


# revision 4
# speedup vs baseline: 2.8056x; 2.8056x over previous
"""CCN promotion layer (gnn_message_passing) for 8x Trainium2 NeuronCores.

reference:
    new_parts = sort(neigh, axis=1)                      # [N, D]
    chi[n,c,a] = (new_parts[n,a] == neigh[n,c])          # [N, D, D]
    feat[n,c,s] = tensors[neigh[n,c], s]                 # [N, D, F]
    promotions[n,c,a,b,s] = chi[n,c,a]*chi[n,c,b]*feat[n,c,s]   # [N,D,D,D,F]

Because `neigh` rows are sorted (harness fill: sorted_randint), new_parts ==
neigh, so chi is the equality matrix of each row with itself: identity except
for duplicate-value runs.  Hence promotions is ~all zeros, with feat[n,c,:] at
the diagonal (c,c,c) slots, plus S^3 blocks for duplicate runs S (rare:
~0.6% of rows for D=16 draws out of N=20000).

Device strategy (memory-regime roofline = writing the ~983 MB output):
  - shard nodes across 8 cores (2500 nodes/core, 122.9 MB output/core)
  - per core, keep 3 persistent SBUF tiles [125 nodes, 16, 819] zeroed once;
    each loop iteration DMAs the 125x48 feature block in, copies it into the
    diagonal slots (stride 819 = 768+48+3), and DMAs the 125x12288 tile to
    its contiguous 6.14 MB slice of the output.  Only the diagonal slots are
    ever rewritten, so the zeros never need refreshing.
  - host fixes up the rare duplicate rows afterwards (a few KB of writes).
"""

import os
import sys

import numpy as np

N, D, F = 20000, 16, 3
NCORES = 8
NPC = N // NCORES                  # nodes per core: 2500
TILE_P = 128                       # nodes per SBUF tile (partition dim)
ROW = D * D * D * F                # 12288 floats per node
CSTRIDE = D * D * F + D * F + F    # 819: flat stride between (c,c,c,0) slots
NBIGBUF = 3

# Tile start rows.  128 partitions per tile keeps every DMA spread across
# all 16 SDMA engines (non-128 partition counts get split onto a handful
# of engines and run at a fraction of HBM bandwidth).  The last tile is
# shifted back so it stays full-width; the overlapped rows are written
# twice with identical bytes, which is benign.
TILE_STARTS = list(range(0, NPC - TILE_P + 1, TILE_P))
if TILE_STARTS[-1] + TILE_P < NPC:
    TILE_STARTS.append(NPC - TILE_P)

_cache = {}


def _get_program():
    if "nc" in _cache:
        return _cache["nc"]
    if "/opt/trn_rl_repo" not in sys.path:
        sys.path.insert(0, "/opt/trn_rl_repo")
    from concourse import bacc, mybir, tile

    nc = bacc.Bacc("TRN2", target_bir_lowering=False, debug=False,
                   num_devices=NCORES)
    feat = nc.dram_tensor("feat", [NPC, D * F], mybir.dt.float32,
                          kind="ExternalInput").ap()
    out = nc.dram_tensor("out", [NPC, ROW], mybir.dt.float32,
                         kind="ExternalOutput").ap()

    with tile.TileContext(nc) as tc:
        with tc.tile_pool(name="bigpool", bufs=1) as bigpool, \
             tc.tile_pool(name="featpool", bufs=4) as featpool:
            bigs = []
            for i in range(NBIGBUF):
                big = bigpool.tile([TILE_P, D, CSTRIDE], mybir.dt.float32,
                                   name=f"big{i}", tag=f"big{i}")
                eng = nc.vector if i % 2 == 0 else nc.gpsimd
                eng.memset(big[:], 0.0)
                bigs.append(big)
            for t, st in enumerate(TILE_STARTS):
                buf = bigs[t % NBIGBUF]
                fb = featpool.tile([TILE_P, D * F], mybir.dt.float32,
                                   name="fb", tag="fb")
                nc.sync.dma_start(out=fb[:], in_=feat[st:st + TILE_P, :])
                nc.vector.tensor_copy(
                    out=buf[:, :, 0:F],
                    in_=fb.rearrange("p (c s) -> p c s", c=D))
                flat = buf.rearrange("p c r -> p (c r)")[:, :ROW]
                nc.sync.dma_start(out=out[st:st + TILE_P, :], in_=flat)
    nc.compile()
    _cache["nc"] = nc
    return nc


def _host_reference(tensors, neigh):
    new_parts = np.sort(neigh, axis=1)
    chi = (new_parts[:, None, :] == neigh[:, :, None]).astype(np.float32)
    feat = tensors[neigh]
    promotions = (chi[:, :, :, None, None] * chi[:, :, None, :, None]
                  * feat[:, :, None, None, :]).astype(np.float32)
    return promotions, new_parts


def kernel(tensors, neigh):
    tensors = np.ascontiguousarray(np.asarray(tensors), dtype=np.float32)
    neigh_in = np.asarray(neigh)
    neigh_i = np.ascontiguousarray(neigh_in.astype(np.int64))

    sorted_ok = bool((neigh_i[:, 1:] >= neigh_i[:, :-1]).all())
    if (not sorted_ok or tensors.shape != (N, F)
            or neigh_i.shape != (N, D)):
        prom, parts = _host_reference(tensors, neigh_in)
        return prom, parts.astype(neigh_in.dtype)

    feat_flat = tensors[neigh_i].reshape(N, D * F)

    nc = _get_program()
    from concourse.bass_utils import run_bass_kernel_spmd

    in_maps = [{"feat": feat_flat[c * NPC:(c + 1) * NPC]}
               for c in range(NCORES)]
    trace = bool(os.environ.get("PROMO_TRACE"))
    res = run_bass_kernel_spmd(nc, in_maps, core_ids=list(range(NCORES)),
                               trace=trace)
    _cache["last_results"] = res

    promotions = np.empty((N, ROW), dtype=np.float32)
    for c in range(NCORES):
        promotions[c * NPC:(c + 1) * NPC] = res.results[c]["out"]
    promotions = promotions.reshape(N, D, D, D, F)

    # Duplicate-run fixup: for a run S of equal values in the sorted row,
    # promotions[n, c, a, b, :] = tensors[v] for all (c, a, b) in S^3.
    dup_rows = np.nonzero((neigh_i[:, 1:] == neigh_i[:, :-1]).any(axis=1))[0]
    for n in dup_rows:
        row = neigh_i[n]
        vals, starts, counts = np.unique(row, return_index=True,
                                         return_counts=True)
        for v, st, ct in zip(vals, starts, counts):
            if ct >= 2:
                S = np.arange(st, st + ct)
                promotions[n][np.ix_(S, S, S)] = tensors[v]

    new_parts = np.sort(neigh_in, axis=1)
    return promotions, new_parts


# revision 5
# speedup vs baseline: 3.0504x; 1.0873x over previous
"""CCN promotion layer (gnn_message_passing) for 8x Trainium2 NeuronCores.

reference:
    new_parts = sort(neigh, axis=1)                      # [N, D]
    chi[n,c,a] = (new_parts[n,a] == neigh[n,c])          # [N, D, D]
    feat[n,c,s] = tensors[neigh[n,c], s]                 # [N, D, F]
    promotions[n,c,a,b,s] = chi[n,c,a]*chi[n,c,b]*feat[n,c,s]   # [N,D,D,D,F]

Because `neigh` rows are sorted (harness fill: sorted_randint), new_parts ==
neigh, so chi is the equality matrix of each row with itself: identity except
for duplicate-value runs.  Hence promotions is ~all zeros, with feat[n,c,:] at
the diagonal (c,c,c) slots, plus S^3 blocks for duplicate runs S (rare:
~0.6% of rows for D=16 draws out of N=20000).

Device strategy (memory-regime roofline = writing the ~983 MB output):
  - shard nodes across 8 cores (2500 nodes/core, 122.9 MB output/core)
  - per core, keep 3 persistent SBUF tiles [125 nodes, 16, 819] zeroed once;
    each loop iteration DMAs the 125x48 feature block in, copies it into the
    diagonal slots (stride 819 = 768+48+3), and DMAs the 125x12288 tile to
    its contiguous 6.14 MB slice of the output.  Only the diagonal slots are
    ever rewritten, so the zeros never need refreshing.
  - host fixes up the rare duplicate rows afterwards (a few KB of writes).
"""

import os
import sys

import numpy as np

N, D, F = 20000, 16, 3
NCORES = 8
NPC = N // NCORES                  # nodes per core: 2500
TILE_P = 128                       # nodes per SBUF tile (partition dim)
ROW = D * D * D * F                # 12288 floats per node
CSTRIDE = D * D * F + D * F + F    # 819: flat stride between (c,c,c,0) slots
NBIGBUF = 3

# Tile start rows.  128 partitions per tile keeps every DMA spread across
# all 16 SDMA engines (non-128 partition counts get split onto a handful
# of engines and run at a fraction of HBM bandwidth).  The last tile is
# shifted back so it stays full-width; the overlapped rows are written
# twice with identical bytes, which is benign.
TILE_STARTS = list(range(0, NPC - TILE_P + 1, TILE_P))
if TILE_STARTS[-1] + TILE_P < NPC:
    TILE_STARTS.append(NPC - TILE_P)

_cache = {}


def _get_program():
    if "nc" in _cache:
        return _cache["nc"]
    if "/opt/trn_rl_repo" not in sys.path:
        sys.path.insert(0, "/opt/trn_rl_repo")
    from concourse import bacc, mybir, tile

    nc = bacc.Bacc("TRN2", target_bir_lowering=False, debug=False,
                   num_devices=NCORES)
    feat = nc.dram_tensor("feat", [NPC, D * F], mybir.dt.float32,
                          kind="ExternalInput").ap()
    out = nc.dram_tensor("out", [NPC, ROW], mybir.dt.float32,
                         kind="ExternalOutput").ap()

    with tile.TileContext(nc) as tc:
        with tc.tile_pool(name="bigpool", bufs=1) as bigpool, \
             tc.tile_pool(name="featpool", bufs=4) as featpool:
            # big0 is on the critical path: zero it with vector+gpsimd in
            # parallel (gpsimd gets more blocks - it starts earlier).
            # big1/big2 go vector-only so gpsimd is free for feat loads.
            bigs = []
            big0 = bigpool.tile([TILE_P, D, CSTRIDE], mybir.dt.float32,
                                name="big0", tag="big0")
            nc.vector.memset(big0[:, 0:7, :], 0.0)
            nc.gpsimd.memset(big0[:, 7:D, :], 0.0)
            bigs.append(big0)
            for i in (1, 2):
                big = bigpool.tile([TILE_P, D, CSTRIDE], mybir.dt.float32,
                                   name=f"big{i}", tag=f"big{i}")
                nc.vector.memset(big[:], 0.0)
                bigs.append(big)
            for t, st in enumerate(TILE_STARTS):
                buf = bigs[t % NBIGBUF]
                fb = featpool.tile([TILE_P, D * F], mybir.dt.float32,
                                   name="fb", tag="fb")
                # Feat loads ride SWDGE so their data never queues behind
                # the big HWDGE output writes (which would stall the diag
                # copies by ~3 tiles).  Tile 0's load goes on HWDGE: the
                # queue is still empty and gpsimd is busy zeroing big0.
                dma_eng = nc.sync if t == 0 else nc.gpsimd
                dma_eng.dma_start(out=fb[:], in_=feat[st:st + TILE_P, :])
                # Diagonal copy on the otherwise-idle scalar engine (ACT):
                # keeps it off the vector queue behind the memsets.
                nc.scalar.copy(
                    out=buf[:, :, 0:F],
                    in_=fb.rearrange("p (c s) -> p c s", c=D))
                flat = buf.rearrange("p c r -> p (c r)")[:, :ROW]
                nc.sync.dma_start(out=out[st:st + TILE_P, :], in_=flat)
    nc.compile()
    _cache["nc"] = nc
    return nc


def _host_reference(tensors, neigh):
    new_parts = np.sort(neigh, axis=1)
    chi = (new_parts[:, None, :] == neigh[:, :, None]).astype(np.float32)
    feat = tensors[neigh]
    promotions = (chi[:, :, :, None, None] * chi[:, :, None, :, None]
                  * feat[:, :, None, None, :]).astype(np.float32)
    return promotions, new_parts


def kernel(tensors, neigh):
    tensors = np.ascontiguousarray(np.asarray(tensors), dtype=np.float32)
    neigh_in = np.asarray(neigh)
    neigh_i = np.ascontiguousarray(neigh_in.astype(np.int64))

    sorted_ok = bool((neigh_i[:, 1:] >= neigh_i[:, :-1]).all())
    if (not sorted_ok or tensors.shape != (N, F)
            or neigh_i.shape != (N, D)):
        prom, parts = _host_reference(tensors, neigh_in)
        return prom, parts.astype(neigh_in.dtype)

    feat_flat = tensors[neigh_i].reshape(N, D * F)

    nc = _get_program()
    from concourse.bass_utils import run_bass_kernel_spmd

    in_maps = [{"feat": feat_flat[c * NPC:(c + 1) * NPC]}
               for c in range(NCORES)]
    trace = bool(os.environ.get("PROMO_TRACE"))
    res = run_bass_kernel_spmd(nc, in_maps, core_ids=list(range(NCORES)),
                               trace=trace)
    _cache["last_results"] = res

    promotions = np.empty((N, ROW), dtype=np.float32)
    for c in range(NCORES):
        promotions[c * NPC:(c + 1) * NPC] = res.results[c]["out"]
    promotions = promotions.reshape(N, D, D, D, F)

    # Duplicate-run fixup: for a run S of equal values in the sorted row,
    # promotions[n, c, a, b, :] = tensors[v] for all (c, a, b) in S^3.
    dup_rows = np.nonzero((neigh_i[:, 1:] == neigh_i[:, :-1]).any(axis=1))[0]
    for n in dup_rows:
        row = neigh_i[n]
        vals, starts, counts = np.unique(row, return_index=True,
                                         return_counts=True)
        for v, st, ct in zip(vals, starts, counts):
            if ct >= 2:
                S = np.arange(st, st + ct)
                promotions[n][np.ix_(S, S, S)] = tensors[v]

    new_parts = np.sort(neigh_in, axis=1)
    return promotions, new_parts
